# revision 8
# baseline (speedup 1.0000x reference)
"""Trainium2 Bass kernel for nn_AttentionHead (B=8, T=4096, D=512, d_k=d_v=64).

Strategy: pure data parallelism — one batch element per NeuronCore (8 cores).
Masked keys (~half) are compacted away on the host (exactly 0 contribution to
softmax numerator and denominator), zero-padded to a 512 multiple TK.

Per core:
  QT2[128,T]   = Wq^T q^T replicated into both partition halves
  KT2[128,TK/2]= Wk^T k^T, even chunks in partitions 0-63, odd in 64-127
  VE [128,nj,65]= [v @ Wv * m, m]  (mask zeroes dead key rows; col 64 = 0/1
                                    mask -> softmax denominator row)
  Phase 2 runs entirely in 64x128 row-tiled PE mode (2 concurrent tiles):
    ST pair p: chunks (2p,2p+1) computed concurrently into a 6-bank PSUM ring
    PT = exp(ST/sqrt(512)) over 3-bank batches (ScalarE, the bottleneck)
    OTe += VE^T @ PT split into j-halves (T0/T8) -> two accumulators
  O = (OT0+OT1)[:64] * 1/(OT0+OT1)[64]  (reciprocal + ones-matmul broadcast)
Host transposes q/k/v shards in, output [64,T] back out.
"""

import sys

import numpy as np

sys.path.insert(0, "/opt/trn_rl_repo")

import concourse.bass as bass  # noqa: F401  (engine namespaces live on nc)
import concourse.mybir as mybir
import concourse.tile as tile
from concourse import bacc
from concourse.bass_utils import run_bass_kernel_spmd

B, T, D, DK = 8, 4096, 512, 64
N_CORES = 8
F32 = mybir.dt.float32
BF16 = mybir.dt.bfloat16
EXP = mybir.ActivationFunctionType.Exp
SCALE = 1.0 / float(np.sqrt(512.0))

_NC_CACHE: dict[int, object] = {}

RING = 6          # ST PSUM ring: 6 banks of [128, 512]
EXPB = 3          # exp batch: 3 adjacent ring banks -> [128, 1536] activation


def _build(tk: int):
    """Build + compile the per-core graph for TK key positions."""
    nj = tk // 128   # 128-row key chunks
    ntb = tk // 512  # 512-col blocks of kT/vT
    nib = T // 512   # query i-blocks
    npair = nj // 2

    nc = bacc.Bacc(None, target_bir_lowering=False)

    qT = nc.declare_dram_parameter("qT", [D, T], BF16, isOutput=False)
    kT = nc.declare_dram_parameter("kT", [D, tk], BF16, isOutput=False)
    vT = nc.declare_dram_parameter("vT", [D, tk], BF16, isOutput=False)
    wq = nc.declare_dram_parameter("wq", [4, 128, DK], BF16, isOutput=False)
    wk = nc.declare_dram_parameter("wk", [4, 128, DK], BF16, isOutput=False)
    wv = nc.declare_dram_parameter("wv", [4, 128, DK], BF16, isOutput=False)
    m01 = nc.declare_dram_parameter("m01", [128, nj], F32, isOutput=False)
    out = nc.declare_dram_parameter("out", [DK, T], F32, isOutput=True)

    with tile.TileContext(nc) as tc:
        with tc.tile_pool(name="const", bufs=1) as constp:
            wq_sb = constp.tile([128, 4, DK], BF16, tag="wq")
            wk_sb = constp.tile([128, 4, DK], BF16, tag="wk")
            wv_sb = constp.tile([128, 4, DK], BF16, tag="wv")
            m01_sb = constp.tile([128, nj], F32, tag="m01")
            ones64 = constp.tile([1, DK], BF16, tag="ones")
            QT2 = constp.tile([128, T], BF16, tag="QT2")
            KTa = constp.tile([DK, tk], BF16, tag="KTa")
            KT2 = constp.tile([128, npair * 128], BF16, tag="KT2")
            VE = constp.tile([128, nj, DK + 1], BF16, tag="VE")

            for c in range(4):
                nc.sync.dma_start(wq_sb[:, c, :], wq[c, :, :])
                nc.sync.dma_start(wk_sb[:, c, :], wk[c, :, :])
                nc.sync.dma_start(wv_sb[:, c, :], wv[c, :, :])
            nc.sync.dma_start(m01_sb[:], m01[:, :])
            ones_f32 = constp.tile([1, DK], F32, tag="ones_f32")
            nc.vector.memset(ones_f32[:], 1.0)
            nc.vector.tensor_copy(ones64[:], ones_f32[:])

            # ---------------- phase 1: projections (full 128x128 PE) -------
            with (
                tc.tile_pool(name="stage", bufs=12) as stage,
                tc.tile_pool(name="psproj", bufs=4, space="PSUM") as psproj,
            ):
                for t in range(ntb):
                    kch = [
                        stage.tile([128, 512], BF16, tag="stg", name=f"kch{t}_{c}")
                        for c in range(4)
                    ]
                    for c in range(4):
                        nc.sync.dma_start(
                            kch[c][:], kT[c * 128:(c + 1) * 128, t * 512:(t + 1) * 512]
                        )
                    ps = psproj.tile([DK, 512], F32, tag="pp")
                    for c in range(4):
                        nc.tensor.matmul(
                            ps[:], wk_sb[:, c, :], kch[c][:],
                            start=(c == 0), stop=(c == 3),
                        )
                    nc.vector.tensor_copy(KTa[:, t * 512:(t + 1) * 512], ps[:])

                    vch = [
                        stage.tile([128, 512], BF16, tag="stg", name=f"vch{t}_{c}")
                        for c in range(4)
                    ]
                    for c in range(4):
                        nc.sync.dma_start(
                            vch[c][:], vT[c * 128:(c + 1) * 128, t * 512:(t + 1) * 512]
                        )
                    for j in range(4):
                        jt = t * 4 + j
                        psv = psproj.tile([128, DK], F32, tag="pp")
                        for c in range(4):
                            nc.tensor.matmul(
                                psv[:],
                                vch[c][:, j * 128:(j + 1) * 128],
                                wv_sb[:, c, :],
                                start=(c == 0), stop=(c == 3),
                            )
                        # zero masked key rows; col 64 = 0/1 mask (denominator)
                        nc.vector.tensor_scalar_mul(
                            VE[:, jt, 0:DK], psv[:], m01_sb[:, jt:jt + 1]
                        )
                        nc.vector.tensor_copy(
                            VE[:, jt, DK:DK + 1], m01_sb[:, jt:jt + 1]
                        )

                for t in range(T // 512):
                    qch = [
                        stage.tile([128, 512], BF16, tag="stg", name=f"qch{t}_{c}")
                        for c in range(4)
                    ]
                    for c in range(4):
                        nc.sync.dma_start(
                            qch[c][:], qT[c * 128:(c + 1) * 128, t * 512:(t + 1) * 512]
                        )
                    ps = psproj.tile([DK, 512], F32, tag="pp")
                    for c in range(4):
                        nc.tensor.matmul(
                            ps[:], wq_sb[:, c, :], qch[c][:],
                            start=(c == 0), stop=(c == 3),
                        )
                    nc.vector.tensor_copy(QT2[0:DK, t * 512:(t + 1) * 512], ps[:])

            # replicate Q^T into partitions 64-127; split K^T into halves
            # (SBUF->SBUF DMA: the only engine that can shift partitions)
            nc.sync.dma_start(QT2[DK:128, :], QT2[0:DK, :])
            ka_pairs = KTa[:, :].rearrange("p (n two c) -> p n two c", two=2, c=128)
            kt2_v = KT2[:, :].rearrange("p (n c) -> p n c", c=128)
            nc.sync.dma_start(kt2_v[0:DK, :, :], ka_pairs[:, :, 0, :])
            nc.sync.dma_start(kt2_v[DK:128, :, :], ka_pairs[:, :, 1, :])

            # ---------------- phase 2: attention (64x128 row-tiled PE) -----
            o_all = constp.tile([DK + 1, T], F32, tag="o_all")
            with (
                tc.tile_pool(name="pt", bufs=3) as ptp,
                tc.tile_pool(name="outp", bufs=2) as outp,
                tc.tile_pool(name="ps_ring", bufs=1, space="PSUM") as ps_ring,
                tc.tile_pool(name="ps_om", bufs=2, space="PSUM") as ps_om,
            ):
                st_ring = ps_ring.tile([128, RING * 512], F32, tag="ring")

                # exp batches of EXPB adjacent ring banks; last batch ragged
                batches = []
                c0 = 0
                while c0 < nj:
                    n = min(EXPB, nj - c0)
                    if (c0 % RING) + n > RING:
                        n = RING - (c0 % RING)
                    batches.append((c0, n))
                    c0 += n

                for ib in range(nib):
                    isl = slice(ib * 512, (ib + 1) * 512)
                    ot0 = ps_om.tile([DK + 1, 512], F32, tag="om", name=f"ot0_{ib}")
                    ot1 = ps_om.tile([DK + 1, 512], F32, tag="om", name=f"ot1_{ib}")
                    pts = {}

                    def emit_st_pair(p):
                        for half, jc in ((0, 2 * p), (1, 2 * p + 1)):
                            s = jc % RING
                            nc.tensor.matmul(
                                st_ring[:, s * 512:(s + 1) * 512],
                                KT2[half * DK:(half + 1) * DK,
                                    p * 128:(p + 1) * 128],
                                QT2[half * DK:(half + 1) * DK, isl],
                                start=True, stop=True,
                                tile_position=(half * DK, 0),
                            )

                    def emit_exp(bi):
                        c0, n = batches[bi]
                        s = c0 % RING
                        pt = ptp.tile(
                            [128, n * 512], BF16, tag="pt", name=f"pt{ib}_{bi}"
                        )
                        nc.scalar.activation(
                            pt[:], st_ring[:, s * 512:(s + n) * 512], EXP,
                            scale=SCALE,
                        )
                        pts[bi] = pt

                    def emit_ot(bi):
                        c0, n = batches[bi]
                        pt = pts.pop(bi)
                        for ci in range(n):
                            jc = c0 + ci
                            psl = slice(ci * 512, (ci + 1) * 512)
                            first = jc == 0
                            last = jc == nj - 1
                            nc.tensor.matmul(
                                ot0[:], VE[0:DK, jc, :], pt[0:DK, psl],
                                start=first, stop=last,
                                skip_group_check=True, tile_position=(0, 0),
                            )
                            nc.tensor.matmul(
                                ot1[:], VE[DK:128, jc, :], pt[DK:128, psl],
                                start=first, stop=last,
                                skip_group_check=True, tile_position=(DK, 0),
                            )

                    nb = len(batches)
                    bi_exp = 0
                    bi_ot = 0
                    filled = 0
                    for p in range(npair):
                        emit_st_pair(p)
                        filled += 2
                        while bi_exp < nb and (
                            batches[bi_exp][0] + batches[bi_exp][1] <= filled
                        ):
                            emit_exp(bi_exp)
                            bi_exp += 1
                            if bi_ot < bi_exp - 1:
                                emit_ot(bi_ot)
                                bi_ot += 1
                    while bi_ot < nb:
                        emit_ot(bi_ot)
                        bi_ot += 1

                    # merge the two half-accumulators; row 64 = denominator
                    ot1_sb = outp.tile([DK + 1, 512], F32, tag="o1s", name=f"o1s{ib}")
                    nc.vector.tensor_copy(ot1_sb[:], ot1[:])
                    nc.vector.tensor_add(o_all[:, isl], ot0[:], ot1_sb[:])

            # tail: broadcast 1/rowsum across partitions and normalize
            with (
                tc.tile_pool(name="tailp", bufs=2) as tailp,
                tc.tile_pool(name="ps_tail", bufs=2, space="PSUM") as ps_tail,
            ):
                for ib in range(nib):
                    isl = slice(ib * 512, (ib + 1) * 512)
                    recip = tailp.tile([1, 512], BF16, tag="rc", name=f"rc{ib}")
                    with nc.allow_low_precision("bf16 broadcast rhs"):
                        nc.vector.reciprocal(recip[:], o_all[DK:DK + 1, isl])
                    bc = ps_tail.tile([DK, 512], F32, tag="bc", name=f"bc{ib}")
                    nc.tensor.matmul(
                        bc[:], ones64[:], recip[:], start=True, stop=True
                    )
                    o = tailp.tile([DK, 512], F32, tag="o", name=f"o{ib}")
                    nc.vector.tensor_mul(o[:], o_all[0:DK, isl], bc[:])
                    nc.sync.dma_start(out[:, isl], o[:])

    nc.compile()
    return nc


def _get_nc(tk: int):
    if tk not in _NC_CACHE:
        _NC_CACHE[tk] = _build(tk)
    return _NC_CACHE[tk]


def _prep_in_maps(k, v, q, pad_mask, Wk, Wq, Wv, tk: int, keep_idx):
    """Per-core shard prep. Keys are compacted to the unmasked positions
    (masked keys contribute exactly 0 to softmax numerator and denominator),
    zero-padded up to tk; m01 marks live rows."""
    import ml_dtypes

    bf16 = ml_dtypes.bfloat16
    wq_r = np.ascontiguousarray(Wq.reshape(4, 128, DK)).astype(bf16)
    wk_r = np.ascontiguousarray(Wk.reshape(4, 128, DK)).astype(bf16)
    wv_r = np.ascontiguousarray(Wv.reshape(4, 128, DK)).astype(bf16)
    in_maps = []
    for b in range(B):
        idx = keep_idx[b]
        n = len(idx)
        kc = np.zeros((tk, D), np.float32)
        vc = np.zeros((tk, D), np.float32)
        kc[:n] = k[b][idx]
        vc[:n] = v[b][idx]
        m = np.zeros(tk, np.float32)
        m[:n] = 1.0
        in_maps.append(
            {
                "qT": np.ascontiguousarray(q[b].T).astype(bf16),
                "kT": np.ascontiguousarray(kc.T).astype(bf16),
                "vT": np.ascontiguousarray(vc.T).astype(bf16),
                "wq": wq_r,
                "wk": wk_r,
                "wv": wv_r,
                "m01": np.ascontiguousarray(m.reshape(tk // 128, 128).T),
            }
        )
    return in_maps


def _run(k, v, q, pad_mask, Wk, Wq, Wv, trace=False, **spmd_kwargs):
    keep_idx = [np.flatnonzero(pad_mask[b, 0] != 1) for b in range(B)]
    max_keep = max(len(i) for i in keep_idx)
    tk = max(512, -(-max_keep // 512) * 512)  # round up to 512-multiple
    nc = _get_nc(tk)
    in_maps = _prep_in_maps(k, v, q, pad_mask, Wk, Wq, Wv, tk, keep_idx)
    res = run_bass_kernel_spmd(
        nc, in_maps, core_ids=list(range(N_CORES)), trace=trace, **spmd_kwargs
    )
    outs = np.stack(
        [np.asarray(res.results[b]["out"]).T for b in range(B)], axis=0
    )
    return outs.astype(np.float32), res


def kernel(k, v, q, pad_mask, Wk, Wq, Wv):
    outs, _ = _run(k, v, q, pad_mask, Wk, Wq, Wv, trace=False)
    return outs


# revision 9
# speedup vs baseline: 1.0509x; 1.0509x over previous
"""Trainium2 Bass kernel for nn_AttentionHead (B=8, T=4096, D=512, d_k=d_v=64).

Strategy: pure data parallelism — one batch element per NeuronCore (8 cores).
Masked keys (~half) are compacted away on the host (exactly 0 contribution to
softmax numerator and denominator), zero-padded to a 512 multiple TK.

Per core:
  QT2[128,T]   = Wq^T q^T replicated into both partition halves
  KT2[128,TK/2]= Wk^T k^T, even chunks in partitions 0-63, odd in 64-127
  VE [128,nj,65]= [v @ Wv * m, m]  (mask zeroes dead key rows; col 64 = 0/1
                                    mask -> softmax denominator row)
  Phase 2 runs entirely in 64x128 row-tiled PE mode (2 concurrent tiles):
    ST pair p: chunks (2p,2p+1) computed concurrently into a 6-bank PSUM ring
    PT = exp(ST/sqrt(512)) over 3-bank batches (ScalarE, the bottleneck)
    OTe += VE^T @ PT split into j-halves (T0/T8) -> two accumulators
  O = (OT0+OT1)[:64] * 1/(OT0+OT1)[64]  (reciprocal + ones-matmul broadcast)
Host transposes q/k/v shards in, output [64,T] back out.
"""

import sys

import numpy as np

sys.path.insert(0, "/opt/trn_rl_repo")

import concourse.bass as bass  # noqa: F401  (engine namespaces live on nc)
import concourse.mybir as mybir
import concourse.tile as tile
from concourse import bacc
from concourse.bass_utils import run_bass_kernel_spmd

B, T, D, DK = 8, 4096, 512, 64
N_CORES = 8
F32 = mybir.dt.float32
BF16 = mybir.dt.bfloat16
EXP = mybir.ActivationFunctionType.Exp
SCALE = 1.0 / float(np.sqrt(512.0))

_NC_CACHE: dict[int, object] = {}

RING = 6          # ST PSUM ring: 6 banks of [128, 512]
EXPB = 3          # exp batch: 3 adjacent ring banks -> [128, 1536] activation


def _build(tk: int):
    """Build + compile the per-core graph for TK key positions."""
    nj = tk // 128   # 128-row key chunks
    ntb = tk // 512  # 512-col blocks of kT/vT
    nib = T // 512   # query i-blocks
    npair = nj // 2

    nc = bacc.Bacc(None, target_bir_lowering=False)

    qT = nc.declare_dram_parameter("qT", [D, T], BF16, isOutput=False)
    kT = nc.declare_dram_parameter("kT", [D, tk], BF16, isOutput=False)
    vT = nc.declare_dram_parameter("vT", [D, tk], BF16, isOutput=False)
    wq = nc.declare_dram_parameter("wq", [4, 128, DK], BF16, isOutput=False)
    wk = nc.declare_dram_parameter("wk", [4, 128, DK], BF16, isOutput=False)
    wv = nc.declare_dram_parameter("wv", [4, 128, DK], BF16, isOutput=False)
    m01 = nc.declare_dram_parameter("m01", [128, nj], F32, isOutput=False)
    out = nc.declare_dram_parameter("out", [DK, T], F32, isOutput=True)

    with tile.TileContext(nc) as tc:
        with tc.tile_pool(name="const", bufs=1) as constp:
            wq_sb = constp.tile([128, 4, DK], BF16, tag="wq")
            wk_sb = constp.tile([128, 4, DK], BF16, tag="wk")
            wv_sb = constp.tile([128, 4, DK], BF16, tag="wv")
            m01_sb = constp.tile([128, nj], F32, tag="m01")
            ones64 = constp.tile([1, DK], BF16, tag="ones")
            QT2 = constp.tile([128, T], BF16, tag="QT2")
            KTa = constp.tile([DK, tk], BF16, tag="KTa")
            KT2 = constp.tile([128, npair * 128], BF16, tag="KT2")
            VE = constp.tile([128, nj, DK + 1], BF16, tag="VE")

            for c in range(4):
                nc.sync.dma_start(wq_sb[:, c, :], wq[c, :, :])
                nc.sync.dma_start(wk_sb[:, c, :], wk[c, :, :])
                nc.sync.dma_start(wv_sb[:, c, :], wv[c, :, :])
            nc.sync.dma_start(m01_sb[:], m01[:, :])
            ones_f32 = constp.tile([1, DK], F32, tag="ones_f32")
            nc.vector.memset(ones_f32[:], 1.0)
            nc.vector.tensor_copy(ones64[:], ones_f32[:])

            # ---------------- phase 1: projections (full 128x128 PE) -------
            # whole-row chunk tiles: 5-8KB contiguous per partition per DMA
            with (
                tc.tile_pool(name="stage", bufs=1) as stage,
                tc.tile_pool(name="psproj", bufs=4, space="PSUM") as psproj,
            ):
                kst = [
                    stage.tile([128, tk], BF16, tag=f"kst{c}", name=f"kst{c}")
                    for c in range(4)
                ]
                vst = [
                    stage.tile([128, tk], BF16, tag=f"vst{c}", name=f"vst{c}")
                    for c in range(4)
                ]
                qst = [
                    stage.tile([128, T], BF16, tag=f"qst{c}", name=f"qst{c}")
                    for c in range(4)
                ]
                for c in range(4):
                    half = tk // 2
                    for h in range(2):
                        hs = slice(h * half, (h + 1) * half)
                        nc.sync.dma_start(kst[c][:, hs], kT[c * 128:(c + 1) * 128, hs])
                        nc.sync.dma_start(vst[c][:, hs], vT[c * 128:(c + 1) * 128, hs])
                    for h in range(2):
                        hs = slice(h * (T // 2), (h + 1) * (T // 2))
                        nc.sync.dma_start(qst[c][:, hs], qT[c * 128:(c + 1) * 128, hs])

                for t in range(ntb):
                    tsl = slice(t * 512, (t + 1) * 512)
                    ps = psproj.tile([DK, 512], F32, tag="pp", name=f"psk{t}")
                    for c in range(4):
                        nc.tensor.matmul(
                            ps[:], wk_sb[:, c, :], kst[c][:, tsl],
                            start=(c == 0), stop=(c == 3),
                        )
                    nc.vector.tensor_copy(KTa[:, tsl], ps[:])

                    for j in range(4):
                        jt = t * 4 + j
                        jsl = slice(jt * 128, (jt + 1) * 128)
                        psv = psproj.tile([128, DK], F32, tag="pp", name=f"psv{jt}")
                        for c in range(4):
                            nc.tensor.matmul(
                                psv[:], vst[c][:, jsl], wv_sb[:, c, :],
                                start=(c == 0), stop=(c == 3),
                            )
                        # zero masked key rows; col 64 = 0/1 mask (denominator)
                        nc.vector.tensor_scalar_mul(
                            VE[:, jt, 0:DK], psv[:], m01_sb[:, jt:jt + 1]
                        )
                        nc.vector.tensor_copy(
                            VE[:, jt, DK:DK + 1], m01_sb[:, jt:jt + 1]
                        )

                for t in range(T // 512):
                    tsl = slice(t * 512, (t + 1) * 512)
                    ps = psproj.tile([DK, 512], F32, tag="pp", name=f"psq{t}")
                    for c in range(4):
                        nc.tensor.matmul(
                            ps[:], wq_sb[:, c, :], qst[c][:, tsl],
                            start=(c == 0), stop=(c == 3),
                        )
                    nc.vector.tensor_copy(QT2[0:DK, tsl], ps[:])

            # replicate Q^T into partitions 64-127; split K^T into halves
            # (SBUF->SBUF DMA: the only engine that can shift partitions)
            nc.sync.dma_start(QT2[DK:128, :], QT2[0:DK, :])
            ka_pairs = KTa[:, :].rearrange("p (n two c) -> p n two c", two=2, c=128)
            kt2_v = KT2[:, :].rearrange("p (n c) -> p n c", c=128)
            nc.sync.dma_start(kt2_v[0:DK, :, :], ka_pairs[:, :, 0, :])
            nc.sync.dma_start(kt2_v[DK:128, :, :], ka_pairs[:, :, 1, :])

            # ---------------- phase 2: attention (64x128 row-tiled PE) -----
            o_all = constp.tile([DK + 1, T], F32, tag="o_all")
            with (
                tc.tile_pool(name="pt", bufs=3) as ptp,
                tc.tile_pool(name="outp", bufs=2) as outp,
                tc.tile_pool(name="ps_ring", bufs=1, space="PSUM") as ps_ring,
                tc.tile_pool(name="ps_om", bufs=2, space="PSUM") as ps_om,
            ):
                st_ring = ps_ring.tile([128, RING * 512], F32, tag="ring")

                # exp batches of EXPB adjacent ring banks; last batch ragged
                batches = []
                c0 = 0
                while c0 < nj:
                    n = min(EXPB, nj - c0)
                    if (c0 % RING) + n > RING:
                        n = RING - (c0 % RING)
                    batches.append((c0, n))
                    c0 += n

                for ib in range(nib):
                    isl = slice(ib * 512, (ib + 1) * 512)
                    ot0 = ps_om.tile([DK + 1, 512], F32, tag="om", name=f"ot0_{ib}")
                    ot1 = ps_om.tile([DK + 1, 512], F32, tag="om", name=f"ot1_{ib}")
                    pts = {}

                    def emit_st_pair(p):
                        for half, jc in ((0, 2 * p), (1, 2 * p + 1)):
                            s = jc % RING
                            nc.tensor.matmul(
                                st_ring[:, s * 512:(s + 1) * 512],
                                KT2[half * DK:(half + 1) * DK,
                                    p * 128:(p + 1) * 128],
                                QT2[half * DK:(half + 1) * DK, isl],
                                start=True, stop=True,
                                tile_position=(half * DK, 0),
                            )

                    def emit_exp(bi):
                        c0, n = batches[bi]
                        s = c0 % RING
                        pt = ptp.tile(
                            [128, n * 512], BF16, tag="pt", name=f"pt{ib}_{bi}"
                        )
                        nc.scalar.activation(
                            pt[:], st_ring[:, s * 512:(s + n) * 512], EXP,
                            scale=SCALE,
                        )
                        pts[bi] = pt

                    def emit_ot(bi):
                        c0, n = batches[bi]
                        pt = pts.pop(bi)
                        for ci in range(n):
                            jc = c0 + ci
                            psl = slice(ci * 512, (ci + 1) * 512)
                            first = jc == 0
                            last = jc == nj - 1
                            nc.tensor.matmul(
                                ot0[:], VE[0:DK, jc, :], pt[0:DK, psl],
                                start=first, stop=last,
                                skip_group_check=True, tile_position=(0, 0),
                            )
                            nc.tensor.matmul(
                                ot1[:], VE[DK:128, jc, :], pt[DK:128, psl],
                                start=first, stop=last,
                                skip_group_check=True, tile_position=(DK, 0),
                            )

                    nb = len(batches)
                    bi_exp = 0
                    bi_ot = 0
                    filled = 0
                    for p in range(npair):
                        emit_st_pair(p)
                        filled += 2
                        while bi_exp < nb and (
                            batches[bi_exp][0] + batches[bi_exp][1] <= filled
                        ):
                            emit_exp(bi_exp)
                            bi_exp += 1
                            if bi_ot < bi_exp - 1:
                                emit_ot(bi_ot)
                                bi_ot += 1
                    while bi_ot < nb:
                        emit_ot(bi_ot)
                        bi_ot += 1

                    # merge the two half-accumulators; row 64 = denominator
                    ot1_sb = outp.tile([DK + 1, 512], F32, tag="o1s", name=f"o1s{ib}")
                    nc.vector.tensor_copy(ot1_sb[:], ot1[:])
                    nc.vector.tensor_add(o_all[:, isl], ot0[:], ot1_sb[:])

            # tail: broadcast 1/rowsum across partitions and normalize
            with (
                tc.tile_pool(name="tailp", bufs=2) as tailp,
                tc.tile_pool(name="ps_tail", bufs=2, space="PSUM") as ps_tail,
            ):
                for ib in range(nib):
                    isl = slice(ib * 512, (ib + 1) * 512)
                    recip = tailp.tile([1, 512], BF16, tag="rc", name=f"rc{ib}")
                    with nc.allow_low_precision("bf16 broadcast rhs"):
                        nc.vector.reciprocal(recip[:], o_all[DK:DK + 1, isl])
                    bc = ps_tail.tile([DK, 512], F32, tag="bc", name=f"bc{ib}")
                    nc.tensor.matmul(
                        bc[:], ones64[:], recip[:], start=True, stop=True
                    )
                    o = tailp.tile([DK, 512], F32, tag="o", name=f"o{ib}")
                    nc.vector.tensor_mul(o[:], o_all[0:DK, isl], bc[:])
                    nc.sync.dma_start(out[:, isl], o[:])

    nc.compile()
    return nc


def _get_nc(tk: int):
    if tk not in _NC_CACHE:
        _NC_CACHE[tk] = _build(tk)
    return _NC_CACHE[tk]


def _prep_in_maps(k, v, q, pad_mask, Wk, Wq, Wv, tk: int, keep_idx):
    """Per-core shard prep. Keys are compacted to the unmasked positions
    (masked keys contribute exactly 0 to softmax numerator and denominator),
    zero-padded up to tk; m01 marks live rows."""
    import ml_dtypes

    bf16 = ml_dtypes.bfloat16
    wq_r = np.ascontiguousarray(Wq.reshape(4, 128, DK)).astype(bf16)
    wk_r = np.ascontiguousarray(Wk.reshape(4, 128, DK)).astype(bf16)
    wv_r = np.ascontiguousarray(Wv.reshape(4, 128, DK)).astype(bf16)
    in_maps = []
    for b in range(B):
        idx = keep_idx[b]
        n = len(idx)
        kc = np.zeros((tk, D), np.float32)
        vc = np.zeros((tk, D), np.float32)
        kc[:n] = k[b][idx]
        vc[:n] = v[b][idx]
        m = np.zeros(tk, np.float32)
        m[:n] = 1.0
        in_maps.append(
            {
                "qT": np.ascontiguousarray(q[b].T).astype(bf16),
                "kT": np.ascontiguousarray(kc.T).astype(bf16),
                "vT": np.ascontiguousarray(vc.T).astype(bf16),
                "wq": wq_r,
                "wk": wk_r,
                "wv": wv_r,
                "m01": np.ascontiguousarray(m.reshape(tk // 128, 128).T),
            }
        )
    return in_maps


def _run(k, v, q, pad_mask, Wk, Wq, Wv, trace=False, **spmd_kwargs):
    keep_idx = [np.flatnonzero(pad_mask[b, 0] != 1) for b in range(B)]
    max_keep = max(len(i) for i in keep_idx)
    tk = max(512, -(-max_keep // 512) * 512)  # round up to 512-multiple
    nc = _get_nc(tk)
    in_maps = _prep_in_maps(k, v, q, pad_mask, Wk, Wq, Wv, tk, keep_idx)
    res = run_bass_kernel_spmd(
        nc, in_maps, core_ids=list(range(N_CORES)), trace=trace, **spmd_kwargs
    )
    outs = np.stack(
        [np.asarray(res.results[b]["out"]).T for b in range(B)], axis=0
    )
    return outs.astype(np.float32), res


def kernel(k, v, q, pad_mask, Wk, Wq, Wv):
    outs, _ = _run(k, v, q, pad_mask, Wk, Wq, Wv, trace=False)
    return outs
